# revision 25
# baseline (speedup 1.0000x reference)
"""Trainium2 kernel for nn_HadamardRotation: y = x @ H, H = 4096x4096 Walsh-Hadamard.

Strategy
--------
H4096 = H64 (x) H64 (Kronecker). Writing d = 64*hi + lo, e = 64*hi' + lo':

    y[r, e] = sum_{hi,lo} H64[lo,lo'] * H64[hi,hi'] * x[r, d]

Two matmul stages with 128-wide contraction (block-diagonal I2 (x) H64
weights), separated by an on-chip "corner turn" (SBUF->SBUF DMA partition
shuffle), all operating in the transposed domain (d on partitions, rows on
the free axis). Host does the cheap transposes / index unscrambles; the
device only ever issues big contiguous DMAs:

  - input:  one 4 MB DMA per slab (contiguous per partition)
  - turn:   one DMA per slab per MC-group; the (t, a) partition-regroup
    collapses to a 3-dim AP because a-count * a-stride == partition pitch
  - output: one DMA per slab per OB-group (contiguous per partition)

FLOPs: 2 * 128/4096 of the naive matmul = 16x reduction.

Data parallel over 8 cores: rows sharded 16384 -> 8 x 2048, weights
replicated. x and Y are staged in HBM as bf16 (host casts).

Layouts (per core, R = 2048 rows, N = 512, SLABS = 4):
  xt  DRAM in  (SLABS, 128, 32*N): xt[s, 64*mu+lo, a*N+n] = x[s*N+n, 128*a+64*mu+lo]
  B1  (128,128): B1[64*mu+lo, 2*lo'+mu]      = H64[lo, lo']
  B2  (128,128): B2[64*nu+32*mu+a, 2*hi'+nu] = H64[2*a+mu, hi']
  stage A (chunk a): u[4c + (2*nu+mu), a*N+n] = sum_k B1[k, .] xt[s, k, a*N+n]
      => holds (hi = 2a+mu, lo' = 2c+nu)
  corner turn:  v[32*t + a, c, n] = u[4*c + t, a*N + n]
  stage B (chunk c): Y[2*hi'+nu, n] = sum_q B2[q, .] v[q, c, n]
      => y[s*N+n, 64*hi' + 2*c + nu]
  Y   DRAM out (SLABS, 32//OB, 128, OB*N):
      Y[s, cb, 2*hi'+nu, j*N+n] = y[s*N+n, 64*hi' + 2*(cb*OB+j) + nu]
"""

import math
import numpy as np
import ml_dtypes

import concourse.bass as bass
import concourse.mybir as mybir
import concourse.tile as tile
from concourse import bacc
from concourse.bass_utils import run_bass_kernel_spmd

N_CORES = 8
DIM = 4096
R_TOTAL = 4 * 4096          # rows after flattening (4, 4096, DIM)
R = R_TOTAL // N_CORES      # rows per core
N = 512                     # free-dim slab (one PSUM bank of fp32)
SLABS = R // N

MODE = "bf16"

CFG = dict(
    ycopy="alt",       # engine(s) for psum->sbuf copy of stage-B out
    ucopy="alt",       # engine(s) for psum->sbuf copy of stage-A out
    turn_eng="scalar",  # corner-turn DMA engine: scalar|sync|gpsimd|rr
    in_eng="sync",
    out_eng="sync",
    merge_c=8,         # c-chunks per corner-turn chunk-group (sb2sb: must be 1)
    out_batch=8,       # c-chunks per output DMA
    pipeline=1,        # emit stage A of slab s+1 before stage B of slab s
    out_bf16=1,        # Y stored bf16 in HBM; host upcasts to f32
    turn_hbm=1,        # corner turn via HBM round-trip (big 3-dim DMAs)
    uout_eng="scalar",  # u -> HBM engine (same ring as vin => ordering)
    vin_eng="scalar",  # turned v <- HBM engine
    xbufs=2, ubufs=2, vbufs=3, ybufs=3,
    # ablation knobs (break correctness; for HW component timing only)
    skip_in=0, skip_a=0, skip_turn=0, skip_b=0, skip_out=0,
)


def _walsh_hadamard64():
    h = np.array([[1.0]], dtype=np.float64)
    while h.shape[0] < 64:
        h = np.block([[h, h], [h, -h]]) / math.sqrt(2.0)
    return h.astype(np.float32)


def _build_weights(H64):
    B1 = np.zeros((128, 128), dtype=np.float32)
    b1v = B1.reshape(2, 64, 64, 2)
    for mu in range(2):
        b1v[mu, :, :, mu] = H64
    B2 = np.zeros((128, 128), dtype=np.float32)
    b2v = B2.reshape(2, 2, 32, 64, 2)
    for nu in range(2):
        for mu in range(2):
            b2v[nu, mu, :, :, nu] = H64[mu::2, :]
    return B1, B2


_NC_CACHE = {}


def _build_bass(mode, loop=0, cfg=None):
    cfg = dict(CFG, **(cfg or {}))
    key = (mode, loop, tuple(sorted(cfg.items())))
    if key in _NC_CACHE:
        return _NC_CACHE[key]

    f32 = mybir.dt.float32
    dt_in = mybir.dt.bfloat16 if mode == "bf16" else f32
    mm_cast = (lambda ap: ap.bitcast(mybir.dt.float32r)) if mode == "fp32r" else (lambda ap: ap)

    OB = cfg["out_batch"]
    MC = cfg["merge_c"]
    NCB = 32 // OB
    dt_out = mybir.dt.bfloat16 if cfg.get("out_bf16") else f32

    nc = bacc.Bacc("TRN2", target_bir_lowering=False, debug=False,
                   num_devices=N_CORES)
    xt_d = nc.dram_tensor("xt", [SLABS, 128, 32 * N], dt_in, kind="ExternalInput")
    B1_d = nc.dram_tensor("B1", [128, 128], dt_in, kind="ExternalInput")
    B2_d = nc.dram_tensor("B2", [128, 128], dt_in, kind="ExternalInput")
    Y_d = nc.dram_tensor("Y", [SLABS, NCB, 128, OB * N], dt_out,
                         kind="ExternalOutput")
    # one staging tensor per turn chunk-group => exact RAW deps (uout_g -> vin_g)
    NG = 32 // MC
    ut_gs = ([nc.dram_tensor(f"uturn{g}", [SLABS, 4 * MC, 32 * N], dt_in,
                             kind="Internal") for g in range(NG)]
             if cfg["turn_hbm"] else None)

    with tile.TileContext(nc) as tc:
        with (
            tc.tile_pool(name="wpool", bufs=1) as wpool,
            tc.tile_pool(name="xpool", bufs=cfg["xbufs"]) as xpool,
            tc.tile_pool(name="upool", bufs=cfg["ubufs"]) as upool,
            tc.tile_pool(name="vpool", bufs=cfg["vbufs"]) as vpool,
            tc.tile_pool(name="ypool", bufs=cfg["ybufs"]) as ypool,
            tc.tile_pool(name="psA", bufs=4, space="PSUM") as psA,
            tc.tile_pool(name="psB", bufs=4, space="PSUM") as psB,
        ):
            B1_sb = wpool.tile([128, 128], dt_in)
            nc.sync.dma_start(B1_sb[:], B1_d[:])
            B2_sb = wpool.tile([128, 128], dt_in)
            nc.sync.dma_start(B2_sb[:], B2_d[:])

            in_eng = getattr(nc, cfg["in_eng"])
            out_eng = getattr(nc, cfg["out_eng"])
            turn_eng = None if cfg["turn_eng"] == "rr" else getattr(nc, cfg["turn_eng"])

            def copy(engine, dst, src, i):
                if engine == "vector":
                    nc.vector.tensor_copy(dst, src)
                elif engine == "scalar":
                    nc.scalar.copy(dst, src)
                elif engine == "alt":
                    if i % 2 == 0:
                        nc.vector.tensor_copy(dst, src)
                    else:
                        nc.scalar.copy(dst, src)
                else:
                    nc.any.tensor_copy(dst, src)

            turn_rr = [nc.scalar, nc.sync, nc.gpsimd]

            def turn(i):
                if cfg["turn_eng"] == "rr":
                    return turn_rr[i % 3]
                return turn_eng

            def phaseA(s):
                if cfg["skip_a"] and cfg["skip_turn"]:
                    u_all = None
                else:
                    u_all = upool.tile([128, 32, N], dt_in)
                    if cfg["skip_a"]:
                        nc.vector.memset(u_all[:, 0, 0:64], 0)
                if not (cfg["skip_in"] and cfg["skip_a"]):
                    xg = xpool.tile([128, 32, N], dt_in)
                    if not cfg["skip_in"]:
                        in_eng.dma_start(xg[:], xt_d[s])
                    if not cfg["skip_a"]:
                        for a in range(32):
                            pu = psA.tile([128, N], f32)
                            nc.tensor.matmul(pu[:], mm_cast(B1_sb[:]),
                                             mm_cast(xg[:, a, :]),
                                             start=True, stop=True)
                            copy(cfg["ucopy"], u_all[:, a, :], pu[:], a)
                return u_all

            def phaseB(s, u_all):
                if u_all is None:
                    return
                ut = u_all.tensor
                PU = u_all.ap[0][0]  # partition stride in elements
                dt_y = dt_out
                vgs = {}

                def get_vc(c):
                    g = c // MC
                    if g not in vgs:
                        vg = vpool.tile([128, MC, N], dt_in)
                        if cfg["turn_hbm"]:
                            if not cfg["skip_turn"]:
                                # stage chunk g (partitions [4MCg, 4MC(g+1)))
                                # to HBM, then read it back turned: flat DRAM
                                # APs have no partition-step limit, so (t, a)
                                # merges into one 128-long stride-N dim.
                                getattr(nc, cfg["uout_eng"]).dma_start(
                                    ut_gs[g][s],
                                    u_all[4 * MC * g:4 * MC * (g + 1), :, :])
                                utt = ut_gs[g][:].tensor
                                base = s * 4 * MC * 32 * N
                                in_ap = bass.AP(utt, base,
                                                [[N, 128], [4 * 32 * N, MC],
                                                 [1, N]])
                                getattr(nc, cfg["vin_eng"]).dma_start(
                                    vg[:], in_ap)
                        else:
                            # sb2sb path: 2 partition dims + 1 free (MC must be 1)
                            in_ap = bass.AP(ut, 4 * g * MC * PU,
                                            [[PU, 4], [N, 32], [1, N]])
                            turn(g).dma_start(vg[:], in_ap)
                        vgs[g] = vg
                    return vgs[g][:, c % MC, :]

                for cb in range(NCB):
                    if cfg["skip_b"]:
                        if not cfg["skip_turn"]:
                            for j in range(OB):
                                get_vc(cb * OB + j)
                        continue
                    yb = ypool.tile([128, OB, N], dt_y)
                    for j in range(OB):
                        c = cb * OB + j
                        vc = get_vc(c) if not cfg["skip_turn"] else None
                        py = psB.tile([128, N], f32)
                        nc.tensor.matmul(py[:], mm_cast(B2_sb[:]),
                                         mm_cast(vc),
                                         start=True, stop=True)
                        copy(cfg["ycopy"], yb[:, j, :], py[:], c)
                    if not cfg["skip_out"]:
                        out_eng.dma_start(Y_d[s, cb], yb[:])

            def body():
                if cfg["pipeline"]:
                    pending = None
                    for s in range(SLABS):
                        u_all = phaseA(s)
                        if pending is not None:
                            phaseB(*pending)
                        pending = (s, u_all)
                    phaseB(*pending)
                else:
                    for s in range(SLABS):
                        phaseB(s, phaseA(s))

            if loop:
                with tc.For_i(0, loop, 1):
                    body()
            else:
                body()

    nc.compile()
    _NC_CACHE[key] = nc
    return nc


def _prep_inputs(x, H, mode, cfg=None):
    cfg = dict(CFG, **(cfg or {}))
    np_in = ml_dtypes.bfloat16 if mode == "bf16" else np.float32
    H64 = (np.asarray(H, dtype=np.float32)[::64, ::64] * 8.0).astype(np.float32)
    B1, B2 = _build_weights(H64)
    B1 = B1.astype(np_in)
    B2 = B2.astype(np_in)
    xf = np.asarray(x, dtype=np.float32).reshape(R_TOTAL, DIM)
    in_maps = []
    for i in range(N_CORES):
        shard = xf[i * R:(i + 1) * R]                     # (R, DIM)
        # xt[s, p, a*N+n] = shard[s*N+n, 128*a + p]
        xt = np.ascontiguousarray(
            shard.reshape(SLABS, N, 32, 128).transpose(0, 3, 2, 1)
        ).astype(np_in).reshape(SLABS, 128, 32 * N)
        in_maps.append({"xt": xt, "B1": B1, "B2": B2})
    return in_maps


def _unscramble(results, cfg=None):
    cfg = dict(CFG, **(cfg or {}))
    OB = cfg["out_batch"]
    NCB = 32 // OB
    outs = []
    for i in range(N_CORES):
        Y = np.asarray(results[i]["Y"])       # (SLABS, NCB, 128, OB*N)
        # Y[s, cb, 2*hi'+nu, j*N+n] = y[s*N+n, 64*hi' + 2*(cb*OB+j) + nu]
        y = (Y.reshape(SLABS, NCB, 64, 2, OB, N)
              .transpose(0, 5, 2, 1, 4, 3)
              .reshape(R, DIM))
        outs.append(y.astype(np.float32))
    return np.concatenate(outs, axis=0).reshape(4, 4096, DIM)


def kernel(x, H, _trace=False, _loop=0, _cfg=None):
    nc = _build_bass(MODE, loop=_loop, cfg=_cfg)
    in_maps = _prep_inputs(x, H, MODE, cfg=_cfg)
    res = run_bass_kernel_spmd(nc, in_maps, core_ids=list(range(N_CORES)),
                               trace=_trace)
    out = _unscramble(res.results, cfg=_cfg)
    if _trace:
        return out, res
    return out


# revision 29
# speedup vs baseline: 1.7983x; 1.7983x over previous
"""Trainium2 kernel for nn_HadamardRotation: y = x @ H, H = 4096x4096 Walsh-Hadamard.

Strategy
--------
H4096 = H64 (x) H64 (Kronecker). Writing d = 64*hi + lo, e = 64*hi' + lo':

    y[r, e] = sum_{hi,lo} H64[lo,lo'] * H64[hi,hi'] * x[r, d]

Two matmul stages with 128-wide contraction (block-diagonal I2 (x) H64
weights), separated by an on-chip "corner turn" (SBUF->SBUF DMA partition
shuffle), all operating in the transposed domain (d on partitions, rows on
the free axis). Host does the cheap transposes / index unscrambles; the
device only ever issues big contiguous DMAs:

  - input:  one 4 MB DMA per slab (contiguous per partition)
  - turn:   one DMA per slab per MC-group; the (t, a) partition-regroup
    collapses to a 3-dim AP because a-count * a-stride == partition pitch
  - output: one DMA per slab per OB-group (contiguous per partition)

FLOPs: 2 * 128/4096 of the naive matmul = 16x reduction.

Data parallel over 8 cores: rows sharded 16384 -> 8 x 2048, weights
replicated. x and Y are staged in HBM as bf16 (host casts).

Layouts (per core, R = 2048 rows, N = 512, SLABS = 4):
  xt  DRAM in  (SLABS, 128, 32*N): xt[s, 64*mu+lo, a*N+n] = x[s*N+n, 128*a+64*mu+lo]
  B1  (128,128): B1[64*mu+lo, 2*lo'+mu]      = H64[lo, lo']
  B2  (128,128): B2[64*nu+32*mu+a, 2*hi'+nu] = H64[2*a+mu, hi']
  stage A (chunk a): u[4c + (2*nu+mu), a*N+n] = sum_k B1[k, .] xt[s, k, a*N+n]
      => holds (hi = 2a+mu, lo' = 2c+nu)
  corner turn:  v[32*t + a, c, n] = u[4*c + t, a*N + n]
  stage B (chunk c): Y[2*hi'+nu, n] = sum_q B2[q, .] v[q, c, n]
      => y[s*N+n, 64*hi' + 2*c + nu]
  Y   DRAM out (SLABS, 32//OB, 128, OB*N):
      Y[s, cb, 2*hi'+nu, j*N+n] = y[s*N+n, 64*hi' + 2*(cb*OB+j) + nu]
"""

import math
import numpy as np
import ml_dtypes

import concourse.bass as bass
import concourse.mybir as mybir
import concourse.tile as tile
from concourse import bacc
from concourse.bass_utils import run_bass_kernel_spmd

N_CORES = 8
DIM = 4096
R_TOTAL = 4 * 4096          # rows after flattening (4, 4096, DIM)
R = R_TOTAL // N_CORES      # rows per core
N = 512                     # free-dim slab (one PSUM bank of fp32)
SLABS = R // N

MODE = "bf16"

CFG = dict(
    ycopy="scalar",    # engine(s) for psum->sbuf copy of stage-B out
    ucopy="vector",    # engine(s) for psum->sbuf copy of stage-A out
    turn_eng="scalar",  # corner-turn DMA engine (sb2sb path): scalar|sync|gpsimd|rr
    in_eng="sync",
    out_eng="sync",
    merge_c=16,        # c-chunks per corner-turn chunk-group (sb2sb: must be 1)
    out_batch=16,      # c-chunks per output DMA
    pipeline=1,        # 1: A(s+1) || B(s); 2: A(s) || T(s-1) || B(s-2)
    out_bf16=1,        # Y stored bf16 in HBM; host upcasts to f32
    turn_hbm=1,        # corner turn via HBM round-trip (big 3-dim DMAs)
    uout_eng="gpsimd",  # u -> HBM engine (Pool ring: no copy contention)
    vin_eng="gpsimd",  # turned v <- HBM engine
    uout_split=4,      # split uout along the a axis (overlaps stage A copies)
    xbufs=2, ubufs=2, vbufs=2, ybufs=2,
    # ablation knobs (break correctness; for HW component timing only)
    skip_in=0, skip_a=0, skip_turn=0, skip_b=0, skip_out=0,
)


def _walsh_hadamard64():
    h = np.array([[1.0]], dtype=np.float64)
    while h.shape[0] < 64:
        h = np.block([[h, h], [h, -h]]) / math.sqrt(2.0)
    return h.astype(np.float32)


def _build_weights(H64):
    B1 = np.zeros((128, 128), dtype=np.float32)
    b1v = B1.reshape(2, 64, 64, 2)
    for mu in range(2):
        b1v[mu, :, :, mu] = H64
    B2 = np.zeros((128, 128), dtype=np.float32)
    b2v = B2.reshape(2, 2, 32, 64, 2)
    for nu in range(2):
        for mu in range(2):
            b2v[nu, mu, :, :, nu] = H64[mu::2, :]
    return B1, B2


_NC_CACHE = {}


def _build_bass(mode, loop=0, cfg=None):
    cfg = dict(CFG, **(cfg or {}))
    key = (mode, loop, tuple(sorted(cfg.items())))
    if key in _NC_CACHE:
        return _NC_CACHE[key]

    f32 = mybir.dt.float32
    dt_in = mybir.dt.bfloat16 if mode == "bf16" else f32
    mm_cast = (lambda ap: ap.bitcast(mybir.dt.float32r)) if mode == "fp32r" else (lambda ap: ap)

    OB = cfg["out_batch"]
    MC = cfg["merge_c"]
    NCB = 32 // OB
    dt_out = mybir.dt.bfloat16 if cfg.get("out_bf16") else f32

    nc = bacc.Bacc("TRN2", target_bir_lowering=False, debug=False,
                   num_devices=N_CORES)
    xt_d = nc.dram_tensor("xt", [SLABS, 128, 32 * N], dt_in, kind="ExternalInput")
    B1_d = nc.dram_tensor("B1", [128, 128], dt_in, kind="ExternalInput")
    B2_d = nc.dram_tensor("B2", [128, 128], dt_in, kind="ExternalInput")
    Y_d = nc.dram_tensor("Y", [SLABS, NCB, 128, OB * N], dt_out,
                         kind="ExternalOutput")
    # one staging tensor per turn chunk-group => exact RAW deps (uout_g -> vin_g)
    NG = 32 // MC
    ut_gs = ([nc.dram_tensor(f"uturn{g}", [SLABS, 4 * MC, 32 * N], dt_in,
                             kind="Internal") for g in range(NG)]
             if cfg["turn_hbm"] else None)

    with tile.TileContext(nc) as tc:
        with (
            tc.tile_pool(name="wpool", bufs=1) as wpool,
            tc.tile_pool(name="xpool", bufs=cfg["xbufs"]) as xpool,
            tc.tile_pool(name="upool", bufs=cfg["ubufs"]) as upool,
            tc.tile_pool(name="vpool", bufs=cfg["vbufs"]) as vpool,
            tc.tile_pool(name="ypool", bufs=cfg["ybufs"]) as ypool,
            tc.tile_pool(name="psA", bufs=4, space="PSUM") as psA,
            tc.tile_pool(name="psB", bufs=4, space="PSUM") as psB,
        ):
            B1_sb = wpool.tile([128, 128], dt_in)
            nc.sync.dma_start(B1_sb[:], B1_d[:])
            B2_sb = wpool.tile([128, 128], dt_in)
            nc.sync.dma_start(B2_sb[:], B2_d[:])

            in_eng = getattr(nc, cfg["in_eng"])
            out_eng = getattr(nc, cfg["out_eng"])
            turn_eng = None if cfg["turn_eng"] == "rr" else getattr(nc, cfg["turn_eng"])

            def copy(engine, dst, src, i):
                if engine == "vector":
                    nc.vector.tensor_copy(dst, src)
                elif engine == "scalar":
                    nc.scalar.copy(dst, src)
                elif engine == "alt":
                    if i % 2 == 0:
                        nc.vector.tensor_copy(dst, src)
                    else:
                        nc.scalar.copy(dst, src)
                else:
                    nc.any.tensor_copy(dst, src)

            turn_rr = [nc.scalar, nc.sync, nc.gpsimd]

            def turn(i):
                if cfg["turn_eng"] == "rr":
                    return turn_rr[i % 3]
                return turn_eng

            def phaseA(s):
                if cfg["skip_a"] and cfg["skip_turn"]:
                    u_all = None
                else:
                    u_all = upool.tile([128, 32, N], dt_in)
                    if cfg["skip_a"]:
                        nc.vector.memset(u_all[:, 0, 0:64], 0)
                if not (cfg["skip_in"] and cfg["skip_a"]):
                    xg = xpool.tile([128, 32, N], dt_in)
                    if not cfg["skip_in"]:
                        in_eng.dma_start(xg[:], xt_d[s])
                    if not cfg["skip_a"]:
                        for a in range(32):
                            pu = psA.tile([128, N], f32)
                            nc.tensor.matmul(pu[:], mm_cast(B1_sb[:]),
                                             mm_cast(xg[:, a, :]),
                                             start=True, stop=True)
                            copy(cfg["ucopy"], u_all[:, a, :], pu[:], a)
                return u_all

            def phaseT(s, u_all):
                # corner turn for slab s: u -> HBM -> (turned) v tiles
                if u_all is None or cfg["skip_turn"]:
                    return None
                ut = u_all.tensor
                PU = u_all.ap[0][0]  # partition stride in elements
                vgs = {}
                for g in range(NG):
                    vg = vpool.tile([128, MC, N], dt_in)
                    if cfg["turn_hbm"]:
                        # stage chunk g (partitions [4MCg, 4MC(g+1))) to HBM,
                        # then read it back turned: flat DRAM APs have no
                        # partition-step limit, so (t, a) merges into one
                        # 128-long stride-N dim.
                        US = cfg["uout_split"]
                        AK = 32 // US
                        for k in range(US):
                            getattr(nc, cfg["uout_eng"]).dma_start(
                                ut_gs[g][s][:, k * AK * N:(k + 1) * AK * N],
                                u_all[4 * MC * g:4 * MC * (g + 1),
                                      k * AK:(k + 1) * AK, :])
                        utt = ut_gs[g][:].tensor
                        base = s * 4 * MC * 32 * N
                        in_ap = bass.AP(utt, base,
                                        [[N, 128], [4 * 32 * N, MC], [1, N]])
                        getattr(nc, cfg["vin_eng"]).dma_start(vg[:], in_ap)
                    else:
                        # sb2sb path: 2 partition dims + 1 free (MC must be 1)
                        in_ap = bass.AP(ut, 4 * g * MC * PU,
                                        [[PU, 4], [N, 32], [1, N]])
                        turn(g).dma_start(vg[:], in_ap)
                    vgs[g] = vg
                return vgs

            def phaseB2(s, vgs):
                if vgs is None or cfg["skip_b"]:
                    return
                for cb in range(NCB):
                    yb = ypool.tile([128, OB, N], dt_out)
                    for j in range(OB):
                        c = cb * OB + j
                        vc = vgs[c // MC][:, c % MC, :]
                        py = psB.tile([128, N], f32)
                        nc.tensor.matmul(py[:], mm_cast(B2_sb[:]),
                                         mm_cast(vc),
                                         start=True, stop=True)
                        copy(cfg["ycopy"], yb[:, j, :], py[:], c)
                    if not cfg["skip_out"]:
                        out_eng.dma_start(Y_d[s, cb], yb[:])

            def body():
                if cfg["pipeline"] == 2:
                    # 3-deep: A(s) || T(s-1) || B(s-2)
                    ua, vv = {}, {}
                    for step in range(SLABS + 2):
                        if step < SLABS:
                            ua[step] = phaseA(step)
                        t = step - 1
                        if 0 <= t < SLABS:
                            vv[t] = phaseT(t, ua.pop(t))
                        b = step - 2
                        if 0 <= b < SLABS:
                            phaseB2(b, vv.pop(b))
                elif cfg["pipeline"]:
                    pending = None
                    for s in range(SLABS):
                        u_all = phaseA(s)
                        if pending is not None:
                            phaseB2(pending[0], phaseT(*pending))
                        pending = (s, u_all)
                    phaseB2(pending[0], phaseT(*pending))
                else:
                    for s in range(SLABS):
                        phaseB2(s, phaseT(s, phaseA(s)))

            if loop:
                with tc.For_i(0, loop, 1):
                    body()
            else:
                body()

    nc.compile()
    _NC_CACHE[key] = nc
    return nc


def _prep_inputs(x, H, mode, cfg=None):
    cfg = dict(CFG, **(cfg or {}))
    np_in = ml_dtypes.bfloat16 if mode == "bf16" else np.float32
    H64 = (np.asarray(H, dtype=np.float32)[::64, ::64] * 8.0).astype(np.float32)
    B1, B2 = _build_weights(H64)
    B1 = B1.astype(np_in)
    B2 = B2.astype(np_in)
    xf = np.asarray(x, dtype=np.float32).reshape(R_TOTAL, DIM)
    in_maps = []
    for i in range(N_CORES):
        shard = xf[i * R:(i + 1) * R]                     # (R, DIM)
        # xt[s, p, a*N+n] = shard[s*N+n, 128*a + p]
        xt = np.ascontiguousarray(
            shard.reshape(SLABS, N, 32, 128).transpose(0, 3, 2, 1)
        ).astype(np_in).reshape(SLABS, 128, 32 * N)
        in_maps.append({"xt": xt, "B1": B1, "B2": B2})
    return in_maps


def _unscramble(results, cfg=None):
    cfg = dict(CFG, **(cfg or {}))
    OB = cfg["out_batch"]
    NCB = 32 // OB
    outs = []
    for i in range(N_CORES):
        Y = np.asarray(results[i]["Y"])       # (SLABS, NCB, 128, OB*N)
        # Y[s, cb, 2*hi'+nu, j*N+n] = y[s*N+n, 64*hi' + 2*(cb*OB+j) + nu]
        y = (Y.reshape(SLABS, NCB, 64, 2, OB, N)
              .transpose(0, 5, 2, 1, 4, 3)
              .reshape(R, DIM))
        outs.append(y.astype(np.float32))
    return np.concatenate(outs, axis=0).reshape(4, 4096, DIM)


def kernel(x, H, _trace=False, _loop=0, _cfg=None):
    nc = _build_bass(MODE, loop=_loop, cfg=_cfg)
    in_maps = _prep_inputs(x, H, MODE, cfg=_cfg)
    res = run_bass_kernel_spmd(nc, in_maps, core_ids=list(range(N_CORES)),
                               trace=_trace)
    out = _unscramble(res.results, cfg=_cfg)
    if _trace:
        return out, res
    return out
